# revision 24
# baseline (speedup 1.0000x reference)
"""CTC loss (keras ctc_batch_cost semantics) on 8 Trainium2 NeuronCores.

Problem: B=512, T=512, C=128 (blank=127), U=64, S=2U+1=129.
loss[b] = -log p(y_true[b] | log_softmax(log(y_pred+eps)))  per sample.

Strategy — packed meet-in-the-middle, pure data parallel (no collectives):
  - Each core owns 64 samples. Partition rows 0..63 run the forward CTC DP
    over t=0..255; rows 64..127 run the *same program* on host-flipped data
    (time-reversed second half, label-reversed y), which by CTC reversal
    symmetry computes the backward DP. Serial DP depth halves to 256 steps
    while every instruction works on 128 partitions.
  - Final combine on-core: one SBUF->SBUF DMA moves the backward rows down,
    then loss = -(logsumexp_s[ln(T@alpha_fwd) + ln rev(alpha_bwd)] + ...).

Numerics: probability-space two-track DP (label track a[u], blank track b[u])
in f32 with (1) scalar max-rescaling and (2) a windowed per-u offset schedule
alpha[u]*e^{-off_j[u]} that recenters the ~5.5 nat/label profile tilt so every
contributing state stays inside f32 range. off_j comes from an f64 numpy
pilot on 64 samples at first call; neighbor-read factors e^{off[u-1]-off[u]}
ride along in the mask tensor (a-track) / a per-window tensor (b-track), so
the inner loop stays at 6 elementwise ops + 1 activation per step.

Emissions: per-sample one-hot gather matmul on PE (labels + blank + ones
column = softmax denominator), p^T via PE transpose, G spilled to DRAM and
reloaded batch-major via strided DMA.
"""

import numpy as np

B_FULL, T_FULL, C, U = 512, 512, 128, 64
NS = 64            # real samples per core
BLK = 128          # partition rows per core (64 fwd + 64 bwd)
TH = T_FULL // 2   # timesteps per core (256)
TC = 128           # chunk size along t
K = 16             # offset/rescale window
EPS = 1e-7
NWIN = TH // K     # 16 windows (index 0 = identity factors)

_CACHE = {}


# ---------------------------------------------------------------------------
# offset schedule: f64 pilot of the same two-track DP, mean ln profile
# ---------------------------------------------------------------------------
def _pilot_schedule(y, p):
    """y:[N,U] int, p:[N,TH,C] f32. Returns off[NWIN,U+1] f64; off[j] is the
    offset vector active during window j (steps (j*K, (j+1)*K]), measured at
    the window's end so mid-window-reachable states carry sane offsets."""
    N = y.shape[0]
    dt = np.float64
    pl = np.take_along_axis(p.astype(dt), y[:, None, :], axis=2) + EPS
    pb = p[:, :, C - 1].astype(dt) + EPS
    m = np.zeros((N, U), dt)
    m[:, 1:] = y[:, 1:] != y[:, :-1]
    a = np.zeros((N, U), dt)
    b = np.zeros((N, U + 1), dt)
    a[:, 0] = pl[:, 0, 0]
    b[:, 0] = pb[:, 0]
    prof = {}
    for t in range(1, TH):
        a_sh = np.concatenate([np.zeros((N, 1), dt), a[:, :-1]], 1)
        a_new = (a + b[:, :U] + m * a_sh) * pl[:, t, :]
        a_sh65 = np.concatenate([np.zeros((N, 1), dt), a], 1)
        b_new = (b + a_sh65) * pb[:, t, None]
        a, b = a_new, b_new
        mx = np.maximum(a.max(1), b.max(1))
        a /= mx[:, None]
        b /= mx[:, None]
        if t % K == 0 or t == TH - 1:
            with np.errstate(divide="ignore"):
                lp = np.log(b)
            lp = np.where(np.isfinite(lp), lp, -500.0)
            v = np.clip(lp.mean(axis=0) - lp.max(axis=1).mean(), -200.0, 0.0)
            for uu in range(1, U + 1):  # cap down-slope: bounded neighbor factors
                v[uu] = max(v[uu], v[uu - 1] - 12.0)
            prof[t] = v
    off = np.zeros((NWIN, U + 1))
    keys = sorted(prof)
    for j in range(1, NWIN):
        t0 = j * K
        nxt = min((kk for kk in keys if kk > t0), default=keys[-1])
        off[j] = prof[nxt]
    return off


def _schedule_tensors(off):
    """Window factor tables (f32, clamped): ra/rb ramps applied entering
    window j, gb neighbor factor and gv mask factor active during window j."""
    ra = np.ones((NWIN, U), np.float64)
    rb = np.ones((NWIN, U + 1), np.float64)
    gb = np.ones((NWIN, U + 1), np.float64)
    gv = np.ones((NWIN, U), np.float64)
    for j in range(1, NWIN):
        offa_o, offa_n = off[j - 1][:U], off[j][:U]
        offb_o, offb_n = off[j - 1], off[j]
        ra[j] = np.exp(offa_o - offa_n)
        rb[j] = np.exp(offb_o - offb_n)
        sh = np.concatenate([[0.0], offa_n[:-1]])
        gv[j] = np.exp(sh - offa_n)
        shb = np.concatenate([[0.0], offa_n])
        gb[j] = np.exp(shb - offb_n)
    clamp = lambda x: np.minimum(x, 1e20).astype(np.float32)
    return clamp(ra), clamp(rb), clamp(gb), clamp(gv)


# ---------------------------------------------------------------------------
# device program
# ---------------------------------------------------------------------------
def _build_bass(off):
    import contextlib

    import concourse.bacc as bacc
    import concourse.mybir as mybir
    from concourse.mybir import AluOpType as ALU
    from concourse.tile import TileContext

    F32 = mybir.dt.float32
    I32 = mybir.dt.int32
    AF = mybir.ActivationFunctionType

    ra, rb, gb, gv = _schedule_tensors(off)
    offa_f, offb_f = off[NWIN - 1][:U], off[NWIN - 1]
    wa = (offa_f + offa_f[::-1]).astype(np.float32)          # [64]
    wb = (offb_f + offb_f[::-1]).astype(np.float32)          # [65]
    # sched rows: j, slot(0=ra,1=rb,2=gb,3=gv), 65
    sched_np = np.zeros((NWIN, 4, U + 1), np.float32)
    sched_np[:, 0, :U] = ra
    sched_np[:, 1, :] = rb
    sched_np[:, 2, :] = gb
    sched_np[:, 3, :U] = gv
    w_np = np.zeros((2, U + 1), np.float32)
    w_np[0, :U] = wa
    w_np[1, :] = wb

    nc = bacc.Bacc("TRN2", target_bir_lowering=False, debug=False, num_devices=8)

    y_in = nc.dram_tensor("y_true", [BLK, U], I32, kind="ExternalInput")
    # host supplies p pre-transposed to [row, c, t] so the gather matmul's
    # moving operand loads directly as [c, t] (no on-device transposes)
    p_in = nc.dram_tensor("y_pred", [BLK, C, TH], F32, kind="ExternalInput")
    loss_out = nc.dram_tensor("loss", [NS, 1], F32, kind="ExternalOutput")

    sched_dram = nc.inline_tensor(sched_np, name="sched")
    w_dram = nc.inline_tensor(w_np, name="wcomb")

    with TileContext(nc) as tc:
        ctx = contextlib.ExitStack()
        with ctx:
            state = ctx.enter_context(tc.tile_pool(name="state", bufs=1))
            hpool = ctx.enter_context(tc.tile_pool(name="hpool", bufs=1))
            tmp = ctx.enter_context(tc.tile_pool(name="tmp", bufs=3))
            ptsb = ctx.enter_context(tc.tile_pool(name="ptsb", bufs=3))
            chunk = ctx.enter_context(tc.tile_pool(name="chunk", bufs=2))
            psumG = ctx.enter_context(tc.tile_pool(name="psumG", bufs=2, space="PSUM"))
            psumS = ctx.enter_context(tc.tile_pool(name="psumS", bufs=1, space="PSUM"))
            dpool = ctx.enter_context(tc.tile_pool(name="dpool", bufs=1, space="DRAM"))
            Gd = dpool.tile([BLK, 66, TH], F32)

            # ---- constants / schedule ----
            sched_sb = state.tile([128, NWIN, 4, U + 1], F32)
            nc.sync.dma_start(
                sched_sb,
                sched_dram.ap().unsqueeze(0).partition_broadcast(128).squeeze(1),
            )
            w_sb = state.tile([128, 2, U + 1], F32)
            nc.sync.dma_start(
                w_sb, w_dram.ap().unsqueeze(0).partition_broadcast(128).squeeze(1)
            )

            eps_ap = state.tile([128, 1], F32)
            nc.vector.memset(eps_ap, EPS)
            ceps_ap = state.tile([128, 1], F32)
            nc.vector.memset(ceps_ap, float(C * EPS))

            # ---- labels, mask ----
            ysb = state.tile([128, U], I32)
            nc.sync.dma_start(ysb, y_in.ap())
            m_base = state.tile([128, U], F32)
            nc.vector.memset(m_base[:, 0:1], 0.0)
            nc.vector.tensor_tensor(
                m_base[:, 1:U], ysb[:, 1:U], ysb[:, 0 : U - 1], ALU.not_equal
            )
            meff = state.tile([128, U], F32)
            nc.vector.tensor_copy(meff, m_base)

            # ---- one-hot gather matrices H[c, row, 66] ----
            iota_c = state.tile([128, 1], F32)
            nc.gpsimd.iota(
                iota_c, pattern=[[0, 1]], base=0, channel_multiplier=1,
                allow_small_or_imprecise_dtypes=True,
            )
            labrep = hpool.tile([128, BLK, 66], F32)
            ybc = y_in.ap().unsqueeze(0).partition_broadcast(128).squeeze(1)
            # split to stay under the 16384-descriptor DMA cap
            nc.gpsimd.dma_start(labrep[0:64, :, 0:U], ybc[0:64])  # casts i32->f32
            nc.gpsimd.dma_start(labrep[64:128, :, 0:U], ybc[64:128])
            nc.vector.memset(labrep[:, :, 64:65], float(C - 1))
            H = hpool.tile([128, BLK, 66], F32)
            nc.vector.tensor_scalar(
                H[:, :, 0:65], labrep[:, :, 0:65], iota_c, None, ALU.is_equal
            )
            nc.vector.memset(H[:, :, 65:66], 1.0)

            # ---- per-row-pair gather matmul -> Gd ----
            # A Matmult encodes at most ONE sync wait, so keep each matmul's
            # cross-engine deps down to its own input DMA: a PE warmup touch
            # absorbs the H-build dep, and a tiny dummy matmul reading the
            # previous Gsb absorbs the PSUM-slot WAR (ScalarE copy) dep.
            scr = psumS.tile([128, 128], F32)
            nc.tensor.matmul(scr[0:66, 0:66], H[:, 0, :], H[:, 0, :],
                             start=True, stop=True)
            gsbs = []
            for b in range(0, BLK, 2):
                pTsb = ptsb.tile([128, 2 * TH], F32)
                nc.sync.dma_start(
                    pTsb,
                    p_in.ap()[b : b + 2].rearrange("s c t -> c s t"),
                )
                if len(gsbs) >= 2:
                    prev = gsbs[-2]
                    nc.tensor.matmul(scr[0:1, 0:1], prev[:, 0:1], prev[:, 0:1],
                                     start=True, stop=True)
                G = psumG.tile([66, 2 * TH], F32)
                # bank-WAW toucher: first write to the reused PSUM slot carries
                # the PE-self wait so the real matmuls keep a single DMA wait
                nc.tensor.matmul(G[:, 0:1], H[:, 0, :], H[:, 0, 0:1],
                                 start=True, stop=True)
                nc.tensor.matmul(G[:, 0:TH], H[:, b, :], pTsb[:, 0:TH],
                                 start=True, stop=True)
                nc.tensor.matmul(G[:, TH : 2 * TH], H[:, b + 1, :],
                                 pTsb[:, TH : 2 * TH], start=True, stop=True)
                Gsb = ptsb.tile([66, 2 * TH], F32, tag="gsb")
                nc.scalar.copy(Gsb, G)
                nc.sync.dma_start(Gd[b], Gsb[:, 0:TH])
                nc.sync.dma_start(Gd[b + 1], Gsb[:, TH : 2 * TH])
                gsbs.append(Gsb)

            # ---- DP state ----
            a0 = state.tile([128, U + 1], F32)  # col0 = zero pad, a[u] at col u+1
            a1 = state.tile([128, U + 1], F32)
            b0 = state.tile([128, U + 1], F32)  # b[u] at col u
            b1 = state.tile([128, U + 1], F32)
            logacc = state.tile([128, 1], F32)
            denacc = state.tile([128, 1], F32)
            for t_ in (a0, a1, b0, b1, logacc, denacc):
                nc.vector.memset(t_, 0.0)
            A = [a0, a1]
            Bt = [b0, b1]
            cur = 0

            # ---- emission chunk loads (+eps) and denominator logs ----
            plcs, pbcs = [], []
            for ci in range(TH // TC):
                tsl = slice(ci * TC, (ci + 1) * TC)
                plc = chunk.tile([128, U, TC], F32, tag="plc")
                nc.sync.dma_start(plc, Gd[:, 0:U, tsl])
                nc.scalar.activation(plc, plc, AF.Identity, bias=eps_ap)
                pbc = chunk.tile([128, TC], F32, tag="pbc")
                nc.sync.dma_start(pbc, Gd[:, 64, tsl])
                nc.scalar.activation(pbc, pbc, AF.Identity, bias=eps_ap)
                denc = chunk.tile([128, TC], F32, tag="denc")
                nc.sync.dma_start(denc, Gd[:, 65, tsl])
                lnden = chunk.tile([128, TC], F32, tag="lnden")
                nc.scalar.activation(lnden, denc, AF.Ln, bias=ceps_ap)
                dsum = tmp.tile([128, 1], F32, tag="dsum")
                nc.vector.reduce_sum(dsum, lnden, axis=mybir.AxisListType.X)
                nc.vector.tensor_add(denacc, denacc, dsum)
                plcs.append(plc)
                pbcs.append(pbc)

            # init from t=0
            nc.vector.tensor_copy(A[0][:, 1:2], plcs[0][:, 0, 0:1])
            nc.vector.tensor_copy(Bt[0][:, 0:1], pbcs[0][:, 0:1])

            # ---- the DP loop ----
            for t in range(1, TH):
                ci, tau = divmod(t, TC)
                plc, pbc = plcs[ci], pbcs[ci]
                j = (t - 1) // K  # factor window active for this step
                nxt = 1 - cur
                Ac, Bc = A[cur], Bt[cur]
                x = tmp.tile([128, U], F32, tag="x")
                nc.vector.tensor_add(x, Ac[:, 1 : U + 1], Bc[:, 0:U])
                y2 = tmp.tile([128, U], F32, tag="y2")
                nc.vector.tensor_tensor(y2, meff, Ac[:, 0:U], ALU.mult)
                x2 = tmp.tile([128, U], F32, tag="x2")
                nc.vector.tensor_add(x2, x, y2)
                nc.vector.tensor_tensor(
                    A[nxt][:, 1 : U + 1], x2, plc[:, :, tau], ALU.mult
                )
                qa = tmp.tile([128, U + 1], F32, tag="qa")
                nc.vector.tensor_tensor(qa, sched_sb[:, j, 2, :], Ac, ALU.mult)
                q = tmp.tile([128, U + 1], F32, tag="q")
                nc.vector.tensor_add(q, Bc, qa)
                nc.scalar.activation(
                    Bt[nxt], q, AF.Copy, scale=pbc[:, tau : tau + 1]
                )
                cur = nxt

                if t % K == 0 and t < TH - 1:
                    jn = t // K  # entering window jn
                    Ac, Bc = A[cur], Bt[cur]
                    nc.vector.tensor_tensor(
                        Ac[:, 1 : U + 1], Ac[:, 1 : U + 1],
                        sched_sb[:, jn, 0, 0:U], ALU.mult,
                    )
                    nc.vector.tensor_tensor(Bc, Bc, sched_sb[:, jn, 1, :], ALU.mult)
                    nc.vector.tensor_tensor(
                        meff, m_base, sched_sb[:, jn, 3, 0:U], ALU.mult
                    )
                    amax = tmp.tile([128, 1], F32, tag="amax")
                    nc.vector.reduce_max(amax, Ac, axis=mybir.AxisListType.X)
                    bmax = tmp.tile([128, 1], F32, tag="bmax")
                    nc.vector.reduce_max(bmax, Bc, axis=mybir.AxisListType.X)
                    mx = tmp.tile([128, 1], F32, tag="mx")
                    nc.vector.tensor_max(mx, amax, bmax)
                    rinv = tmp.tile([128, 1], F32, tag="rinv")
                    nc.vector.reciprocal(rinv, mx)
                    nc.vector.tensor_scalar_mul(Ac, Ac, rinv)
                    nc.vector.tensor_scalar_mul(Bc, Bc, rinv)
                    lg = tmp.tile([128, 1], F32, tag="lg")
                    nc.scalar.activation(lg, mx, AF.Ln)
                    nc.vector.tensor_add(logacc, logacc, lg)

            # ---- epilogue: T-applied fwd state ----
            Ac, Bc = A[cur], Bt[cur]
            Ya = state.tile([128, U], F32)
            nc.vector.tensor_add(Ya, Ac[:, 1 : U + 1], Bc[:, 0:U])
            ya2 = tmp.tile([128, U], F32, tag="x")
            nc.vector.tensor_tensor(ya2, meff, Ac[:, 0:U], ALU.mult)
            nc.vector.tensor_add(Ya, Ya, ya2)
            Yb = state.tile([128, U + 1], F32)
            jl = NWIN - 1
            nc.vector.tensor_tensor(Yb, sched_sb[:, jl, 2, :], Ac, ALU.mult)
            nc.vector.tensor_add(Yb, Yb, Bc)

            # pack raw state + accs, swap bwd rows down via SBUF->SBUF DMA
            pk = state.tile([128, 131], F32)
            nc.vector.tensor_copy(pk[:, 0:U], Ac[:, 1 : U + 1])
            nc.vector.tensor_copy(pk[:, U : U + 65], Bc)
            nc.vector.tensor_copy(pk[:, 129:130], logacc)
            nc.vector.tensor_copy(pk[:, 130:131], denacc)
            swp = state.tile([128, 131], F32)
            nc.sync.dma_start(swp[0:NS, :], pk[NS:BLK, :])

            # ---- combine on rows 0..63 ----
            AFt = AF
            lya = state.tile([128, U], F32)
            nc.scalar.activation(lya[0:NS], Ya[0:NS], AFt.Ln)
            lyb = state.tile([128, U + 1], F32)
            nc.scalar.activation(lyb[0:NS], Yb[0:NS], AFt.Ln)
            lab_ = state.tile([128, U], F32)
            nc.scalar.activation(lab_[0:NS], swp[0:NS, 0:U], AFt.Ln)
            lbb = state.tile([128, U + 1], F32)
            nc.scalar.activation(lbb[0:NS], swp[0:NS, U : U + 65], AFt.Ln)

            za = state.tile([128, U], F32)
            nc.vector.tensor_add(za[0:NS], lya[0:NS], lab_[0:NS, ::-1])
            nc.vector.tensor_add(za[0:NS], za[0:NS], w_sb[0:NS, 0, 0:U])
            zb = state.tile([128, U + 1], F32)
            nc.vector.tensor_add(zb[0:NS], lyb[0:NS], lbb[0:NS, ::-1])
            nc.vector.tensor_add(zb[0:NS], zb[0:NS], w_sb[0:NS, 1, :])

            mza = tmp.tile([128, 1], F32, tag="amax")
            nc.vector.reduce_max(mza[0:NS], za[0:NS], axis=mybir.AxisListType.X)
            mzb = tmp.tile([128, 1], F32, tag="bmax")
            nc.vector.reduce_max(mzb[0:NS], zb[0:NS], axis=mybir.AxisListType.X)
            M = state.tile([128, 1], F32)
            nc.vector.tensor_max(M[0:NS], mza[0:NS], mzb[0:NS])
            nc.vector.tensor_scalar(
                za[0:NS], za[0:NS], M[0:NS], None, mybir.AluOpType.subtract
            )
            nc.vector.tensor_scalar(
                zb[0:NS], zb[0:NS], M[0:NS], None, mybir.AluOpType.subtract
            )
            ea = state.tile([128, U], F32)
            nc.scalar.activation(ea[0:NS], za[0:NS], AFt.Exp)
            eb = state.tile([128, U + 1], F32)
            nc.scalar.activation(eb[0:NS], zb[0:NS], AFt.Exp)
            sa = tmp.tile([128, 1], F32, tag="mx")
            nc.vector.reduce_sum(sa[0:NS], ea[0:NS], axis=mybir.AxisListType.X)
            sb_ = tmp.tile([128, 1], F32, tag="rinv")
            nc.vector.reduce_sum(sb_[0:NS], eb[0:NS], axis=mybir.AxisListType.X)
            d = state.tile([128, 1], F32)
            nc.vector.tensor_add(d[0:NS], sa[0:NS], sb_[0:NS])
            lnd = state.tile([128, 1], F32)
            nc.scalar.activation(lnd[0:NS], d[0:NS], AFt.Ln)

            acc = state.tile([128, 1], F32)
            nc.vector.tensor_add(acc[0:NS], lnd[0:NS], M[0:NS])
            nc.vector.tensor_add(acc[0:NS], acc[0:NS], logacc[0:NS])
            nc.vector.tensor_add(acc[0:NS], acc[0:NS], swp[0:NS, 129:130])
            nc.vector.tensor_sub(acc[0:NS], acc[0:NS], denacc[0:NS])
            nc.vector.tensor_sub(acc[0:NS], acc[0:NS], swp[0:NS, 130:131])
            lossr = state.tile([128, 1], F32)
            nc.vector.tensor_scalar_mul(lossr[0:NS], acc[0:NS], -1.0)
            nc.sync.dma_start(loss_out.ap(), lossr[0:NS])

    nc.compile()
    return nc


# ---------------------------------------------------------------------------
# host wrapper
# ---------------------------------------------------------------------------
def _make_in_maps(y_true, y_pred):
    y_true = np.ascontiguousarray(y_true, dtype=np.int32)
    y_pred = np.ascontiguousarray(y_pred, dtype=np.float32)
    in_maps = []
    for k in range(8):
        sl = slice(NS * k, NS * (k + 1))
        y = y_true[sl]
        p = y_pred[sl]
        yc = np.concatenate([y, y[:, ::-1]], axis=0)
        pc = np.concatenate([p[:, :TH], p[:, TH:][:, ::-1]], axis=0)
        pct = pc.transpose(0, 2, 1)  # [row, c, t] for transpose-free gather
        in_maps.append(
            {
                "y_true": np.ascontiguousarray(yc),
                "y_pred": np.ascontiguousarray(pct),
            }
        )
    return in_maps


def _get_nc(y_true, y_pred):
    if "nc" not in _CACHE:
        yp = np.asarray(y_pred[:NS, :TH], dtype=np.float32)
        yt = np.asarray(y_true[:NS], dtype=np.int32)
        off = _pilot_schedule(yt, yp)
        _CACHE["nc"] = _build_bass(off)
    return _CACHE["nc"]


def _run(y_true, y_pred, trace=False):
    from concourse.bass_utils import run_bass_kernel_spmd

    nc = _get_nc(y_true, y_pred)
    in_maps = _make_in_maps(y_true, y_pred)
    res = run_bass_kernel_spmd(nc, in_maps, core_ids=list(range(8)), trace=trace)
    out = np.zeros((B_FULL, 1), np.float32)
    for k in range(8):
        out[NS * k : NS * (k + 1)] = res.results[k]["loss"]
    return out, res


def kernel(y_true, y_pred):
    out, _ = _run(y_true, y_pred, trace=False)
    return out


def kernel_profiled(y_true, y_pred):
    return _run(y_true, y_pred, trace=True)


def _build_sharded():
    """Replicates bass2jax.run_bass_via_pjrt's jit so we can keep inputs
    device-resident and time steady-state executions."""
    import jax
    import concourse.mybir as mybir
    from concourse import bass2jax
    from jax.sharding import Mesh, PartitionSpec
    from jax.experimental.shard_map import shard_map

    nc = _CACHE["nc"]
    bass2jax.install_neuronx_cc_hook()
    partition_name = nc.partition_id_tensor.name if nc.partition_id_tensor else None
    in_names, out_names, out_avals, zero_outs = [], [], [], []
    for alloc in nc.m.functions[0].allocations:
        if not isinstance(alloc, mybir.MemoryLocationSet):
            continue
        name = alloc.memorylocations[0].name
        if alloc.kind == "ExternalInput":
            if name != partition_name:
                in_names.append(name)
        elif alloc.kind == "ExternalOutput":
            out_names.append(name)
            shape = tuple(alloc.tensor_shape)
            dtype = mybir.dt.np(alloc.dtype)
            out_avals.append(jax.core.ShapedArray(shape, dtype))
            zero_outs.append(np.zeros(shape, dtype))
    n_params = len(in_names)
    in_names_all = in_names + out_names
    if partition_name is not None:
        in_names_all = in_names_all + [partition_name]

    def _body(*args):
        operands = list(args)
        if partition_name is not None:
            operands.append(bass2jax.partition_id_tensor())
        outs = bass2jax._bass_exec_p.bind(
            *operands,
            out_avals=tuple(out_avals),
            in_names=tuple(in_names_all),
            out_names=tuple(out_names),
            lowering_input_output_aliases=(),
            sim_require_finite=True,
            sim_require_nnan=True,
            nc=nc,
        )
        return tuple(outs)

    devices = jax.devices()[:8]
    mesh = Mesh(np.asarray(devices), ("core",))
    n_outs = len(out_names)
    sharded = jax.jit(
        shard_map(
            _body, mesh=mesh,
            in_specs=(PartitionSpec("core"),) * (n_params + n_outs),
            out_specs=(PartitionSpec("core"),) * n_outs,
            check_rep=False,
        ),
        keep_unused=True,
    )
    return sharded, in_names, zero_outs


def benchmark(y_true, y_pred, iters=10):
    """Median wall time (ns) of steady-state executions with device-resident
    inputs; includes PJRT dispatch but not host->device transfer."""
    import time

    import jax

    _get_nc(y_true, y_pred)
    sharded, in_names, zero_outs = _build_sharded()
    in_maps = _make_in_maps(y_true, y_pred)
    concat_in = [
        np.concatenate([in_maps[c][n] for c in range(8)], axis=0) for n in in_names
    ]
    concat_zeros = [
        np.zeros((8 * z.shape[0], *z.shape[1:]), z.dtype) for z in zero_outs
    ]
    dev_in = [jax.device_put(x) for x in concat_in]
    dev_zero = [jax.device_put(z) for z in concat_zeros]
    out = sharded(*dev_in, *dev_zero)  # warmup/compile
    jax.block_until_ready(out)
    times = []
    for _ in range(iters):
        t0 = time.perf_counter()
        out = sharded(*dev_in, *dev_zero)
        jax.block_until_ready(out)
        times.append(time.perf_counter() - t0)
    times.sort()
    return int(times[len(times) // 2] * 1e9)
